# revision 42
# baseline (speedup 1.0000x reference)
"""BasicLSTM (T=8192, IN=H=OUT=1024, batch=1) Trainium2 Bass kernel.

Strategy: the LSTM recurrence is strictly serial in t, and an 8-core
AllGather has a ~4.6us latency floor per step -- far more than the
~0.5us of per-step compute that tensor parallelism over the gate matmul
would save (the sharding hint's TP option was evaluated and rejected on
this ground; batch=1 rules out data parallelism).  So the whole
computation runs on ONE NeuronCore; the surrounding batched matmuls
(input projection X = x @ Wx^T + b over all t, output projection
y = h @ out_w^T + out_b over all t) are ~1.3 ms next to the 8192-step
recurrence (~75 ms measured).

Per-step recurrence (see emit_step):
  - PE matvec: h stationary (M=1 columns, tiny weight loads), W_h^T
    streamed as the bf16 moving operand (1 cycle/row vs fp32's 4),
    split across 4 concurrent PE column groups via tile_position --
    4x the single-stream SBUF->PE ingestion rate.
  - Gates live quarter-major, per-quarter column order [f|i|g|o], and
    the matvec streams three column blocks [f+i][g][o], EACH INTO ITS
    OWN PSUM TILE.  Tile-granular dependencies then let sigmoid(f)/
    sigmoid(i) fire mid-matvec and the whole c = f*c + i*tanh(g) chain
    (DVE, row-land) hide under the g/o streaming; only
    sigmoid(o) -> selector-matmul -> h-mul is exposed after the matvec.
  - The X/bias contribution enters each PSUM tile via K=1 ones-matmuls
    (start=True) issued during the previous step's tail.
  - Row layout is band-replicated (hidden quarter q on partition band
    32q); a band's 31 unused lanes compute finite garbage that the
    selector matmuls multiply by 0.0 (every lane passes through
    sigmoid/tanh or the bounded c update first, so no NaN/Inf).
  - c and sigma(o) return to column-major via tiny selector matmuls
    (lhsT = row tile, rhs = sel with sel[32n, n] = 1, N=4): ~0.1us each
    instead of ~1.3us for four 128x128 PE transposes.  Then
    h_col = sigma(o)_col * tanh(c_col) on [128, 8] directly -- h_col is
    the next step's stationary operand and the stored history row.
  - c stays fp32; weights/h/x are bf16 with fp32 PSUM accumulation
    (measured end-to-end error ~4.4e-3 of output scale, flat in t).

This file also carries two workarounds for the current walrus build,
which accepts only ONE sync-wait per instruction: the TileContext exit
drain is split into one drain per wait, and multi-wait instructions get
their extra waits moved onto no-fuse NOPs on the same engine queue.
"""

import numpy as np
import ml_dtypes

import concourse.bass as bass
import concourse.mybir as mybir
import concourse.tile as tile
from concourse.vector_clock import ScopedClock
from concourse.bass_utils import run_bass_kernel_spmd

def _drain_and_barrier_split(self, tick_clock, wait_clock):
    nc = self.nc
    drain_inst = nc.sync.drain()
    wait_clock.add_sem_waits(
        drain_inst.ins, ScopedClock({None: tick_clock.global_clock})
    )
    si = drain_inst.ins.sync_info
    if si is not None and len(si.on_wait) > 1:
        extra_waits = list(si.on_wait[1:])
        del si.on_wait[1:]
        for w in extra_waits:
            d2 = nc.sync.drain()
            d2.ins.sync_info = mybir.SyncInfo(on_wait=[w], on_update=[])

    nc.all_engine_barrier()
    assert self.sems is not None
    popped = nc._tile_sem_poison_stack.pop()
    assert popped is self._sem_poison
    nc.clear_and_free_semaphores(list(self.sems.allocated().values()))
    nc.all_engine_barrier()


tile.TileContext._drain_and_barrier = _drain_and_barrier_split


# ---------------------------------------------------------------------------
# This walrus build accepts only ONE sync-wait per instruction (setupSyncWait
# "Too many sync wait commands").  Tile's wait assignment freely attaches
# several.  Split: keep one wait on the instruction, move the rest onto
# no-fuse NOPs inserted just before it on the same engine queue.
_orig_lower = tile.TileContext._lower_ordered_insts
_nop_ctr = [0]


def _split_multi_waits(self, ordered):
    for bb_name, insts in ordered.items():
        out = []
        for inst in insts:
            si = getattr(inst, "sync_info", None)
            waits = list(si.on_wait) if si is not None and si.on_wait else []
            if len(waits) > 1 and getattr(inst, "engine", None) is not None:
                extra, keep = waits[:-1], waits[-1:]
                si.on_wait = keep
                for w in extra:
                    _nop_ctr[0] += 1
                    nop = mybir.InstNoOp(
                        name=f"I-waitnop-{_nop_ctr[0]}",
                        ins=[], outs=[],
                        text_hint="split_wait",
                        bass_nofuse=True,
                    )
                    nop.engine = inst.engine
                    nop.sync_info = mybir.SyncInfo(on_wait=[w], on_update=[])
                    out.append(nop)
            out.append(inst)
        insts[:] = out
    return _orig_lower(self, ordered)


tile.TileContext._lower_ordered_insts = _split_multi_waits

F32 = mybir.dt.float32
BF16 = mybir.dt.bfloat16
AF = mybir.ActivationFunctionType

H = 1024          # hidden
IN = 1024         # input
G = 4096          # gates
OUT = 1024
Q = 4             # quarters / col groups
S = 256           # hidden per quarter
KC = 8            # k chunks of 128
NB = 256          # matvec n-block (<= 512)

# permuted gate order within each quarter: f, i, g, o -- f/i early (their
# ACTs + c-partials run under later matvec blocks), g next (c completes
# under the o block), o last (only so -> selector -> mul stays exposed)
_BLK = {"g": 2048, "i": 0, "f": 1024, "o": 3072}
_ORDER = ["f", "i", "g", "o"]


def perm_rows() -> np.ndarray:
    """perm[c] = original W_w row index for permuted gate column c."""
    p = np.zeros(G, dtype=np.int64)
    for q in range(Q):
        for bi, bname in enumerate(_ORDER):
            base = _BLK[bname]
            for u in range(S):
                p[q * 1024 + bi * S + u] = base + q * S + u
    return p


def kcol_of_chunk(j: int) -> int:
    """h_col column index holding hid chunk j (see module docstring)."""
    return (j // 2) if (j % 2 == 0) else (4 + j // 2)


def chunk_of_kcol(j: int) -> int:
    """hid chunk stored in h_col column j (inverse of kcol_of_chunk)."""
    return 2 * j if j < 4 else 2 * (j - 4) + 1


def host_prep(x, W_w, W_b, out_w, out_b, T):
    """numpy-side sharding prep: permute/transpose/cast weights + x."""
    bf = ml_dtypes.bfloat16
    pr = perm_rows()
    x2 = np.ascontiguousarray(x.reshape(T, IN))
    xT = np.ascontiguousarray(x2.T.astype(bf))                    # [IN, T]
    Wp = W_w[pr]                                                  # [G, IN+H] permuted rows
    WxT = np.ascontiguousarray(Wp[:, :IN].T.astype(bf))           # [IN, G]
    WhT = np.ascontiguousarray(Wp[:, IN:].T.astype(bf))           # [H, G]
    bp = np.ascontiguousarray(W_b[pr].astype(bf)).reshape(1, G)   # [1, G]
    owT = np.ascontiguousarray(out_w.T.astype(bf))                # [H, OUT]
    ob = np.ascontiguousarray(out_b.astype(bf)).reshape(1, OUT)
    return {"xT": xT, "WxT": WxT, "WhT": WhT, "bperm": bp,
            "outwT": owT, "outb": ob}


def build_nc(T, BODY=32, use_loop=True, loop_trips=None, outer_rep=1,
             variant='full', FILL_MID=0, FILL_END=0):
    """Build the Bass module. T must be divisible by 128 and BODY.
    loop_trips: override recurrence loop trip count (timing experiments).
    FILL_MID/FILL_END: dummy HAM-warming matmuls per step (see emit_fill)."""
    assert T % 128 == 0 and T % BODY == 0
    nc = bass.Bass("TRN2", detect_race_conditions=False)

    # ---- I/O ----
    xT_h = nc.dram_tensor("xT", [IN, T], BF16, kind="ExternalInput")
    WxT_h = nc.dram_tensor("WxT", [IN, G], BF16, kind="ExternalInput")
    WhT_h = nc.dram_tensor("WhT", [H, G], BF16, kind="ExternalInput")
    bp_h = nc.dram_tensor("bperm", [1, G], BF16, kind="ExternalInput")
    owT_h = nc.dram_tensor("outwT", [H, OUT], BF16, kind="ExternalInput")
    ob_h = nc.dram_tensor("outb", [1, OUT], BF16, kind="ExternalInput")
    Y_h = nc.dram_tensor("Y", [T, OUT], F32, kind="ExternalOutput")
    X_h = nc.dram_tensor("Xc", [T, G], BF16)          # internal scratch
    Hh_h = nc.dram_tensor("Hst", [H, T], BF16)        # internal: h history, [hid, t]

    TT = T // 128  # time tiles

    with tile.TileContext(nc) as tc:
        # ---------------- phase 1: X_contrib ----------------
        with tc.tile_pool(name="p1w", bufs=1) as wpool, \
             tc.tile_pool(name="p1x", bufs=3) as xpool, \
             tc.tile_pool(name="p1o", bufs=4) as opool, \
             tc.tile_pool(name="p1ps", bufs=4, space="PSUM") as pspool, \
             tc.tile_pool(name="p1c", bufs=1) as cpool:
            wx = wpool.tile([128, KC * G], BF16)
            for k in range(KC):
                nc.sync.dma_start(out=wx[:, k * G:(k + 1) * G],
                                  in_=WxT_h[k * 128:(k + 1) * 128, :])
            onescol = cpool.tile([1, 128], BF16)
            nc.vector.memset(onescol, 1.0)
            bsb = cpool.tile([1, G], BF16)
            nc.sync.dma_start(out=bsb, in_=bp_h[:, :])

            for tt in range(TT):
                xk = xpool.tile([128, KC * 128], BF16, tag="xk")
                for k in range(KC):
                    nc.sync.dma_start(
                        out=xk[:, k * 128:(k + 1) * 128],
                        in_=xT_h[k * 128:(k + 1) * 128, tt * 128:(tt + 1) * 128])
                for sl in range(G // 512):
                    ps = pspool.tile([128, 512], F32, tag="ps")
                    nc.tensor.matmul(ps[:, :], onescol[0:1, :],
                                     bsb[0:1, sl * 512:(sl + 1) * 512],
                                     start=True, stop=False)
                    for k in range(KC):
                        nc.tensor.matmul(
                            ps[:, :], xk[:, k * 128:(k + 1) * 128],
                            wx[:, k * G + sl * 512: k * G + (sl + 1) * 512],
                            start=False, stop=(k == KC - 1))
                    ob_t = opool.tile([128, 512], BF16, tag="ob")
                    nc.vector.tensor_copy(ob_t[:, :], ps[:, :])
                    nc.sync.dma_start(
                        out=X_h[tt * 128:(tt + 1) * 128, sl * 512:(sl + 1) * 512],
                        in_=ob_t[:, :])

        # ---------------- phase 2: recurrence ----------------
        RING = BODY          # X ring steps held in SBUF (partitions 0,32,64,96)
        X_q = X_h.rearrange("t (q n) -> q t n", q=4)       # [4, T, 1024]
        Hh_v = Hh_h.rearrange("(j p) t -> p j t", p=128)   # [128, 8, T]

        with tc.tile_pool(name="p2w", bufs=1) as wpool, \
             tc.tile_pool(name="p2st", bufs=1) as st, \
             tc.tile_pool(name="p2x", bufs=1) as xr, \
             tc.tile_pool(name="p2hr", bufs=2) as hrp, \
             tc.tile_pool(name="p2sc", bufs=2) as sc, \
             tc.tile_pool(name="p2ps", bufs=2, space="PSUM") as psg, \
             tc.tile_pool(name="p2pt", bufs=1, space="PSUM") as pst:
            wh = wpool.tile([128, KC * G], BF16)
            for k in range(KC):
                nc.sync.dma_start(out=wh[:, k * G:(k + 1) * G],
                                  in_=WhT_h[k * 128:(k + 1) * 128, :])
            ones32 = st.tile([128, 32], BF16)
            nc.vector.memset(ones32, 1.0)
            h_col = st.tile([128, 8], BF16)
            nc.vector.memset(h_col, 0.0)

            # Only partition 32q of each band carries real data (M=1 matmul
            # outputs); the other 31 lanes of every row-land op compute
            # garbage.  That garbage must stay FINITE (the selector matmul
            # multiplies it by 0.0: 0*Inf/NaN would poison whole columns),
            # which holds because every lane goes through sigmoid/tanh before
            # reaching the selector-matmul input -- provided the initial
            # PSUM/SBUF contents are defined.  One-time memsets below
            # guarantee that.
            # sfsi = [sig(f) | sig(i)], ctg = [c | tanh(g)], uv = [f*c | i*g]
            # -- paired halves so one wide ACT/DVE op covers two gates.
            sfsi = st.tile([128, 2 * S], F32)
            ctg = st.tile([128, 2 * S], F32)
            nc.vector.memset(ctg, 0.0)
            uv = st.tile([128, 2 * S], F32)
            so = st.tile([128, S], BF16)
            th = st.tile([128, S], BF16)
            hm = st.tile([128, S], BF16)
            # selector: sel[32n, n] = 1; hm^T @ sel picks row 32n into col n,
            # i.e. a 128x4 transpose of the 4 band rows for the price of one
            # small stationary load + an N=4 matmul (vs a full 128x128
            # PE-transpose per band pair).
            sel = st.tile([128, 4], BF16)
            nc.vector.memset(sel, 0.0)
            self32 = st.tile([128, 4], F32)
            nc.vector.memset(self32, 0.0)
            for n in range(4):
                nc.vector.memset(sel[32 * n:32 * n + 1, n:n + 1], 1.0)
                nc.vector.memset(self32[32 * n:32 * n + 1, n:n + 1], 1.0)
            thc = st.tile([128, 8], BF16)
            zero1 = st.tile([1, 1], BF16)
            nc.vector.memset(zero1, 0.0)
            h_alt = st.tile([128, 8], BF16)   # t_selo: sink for severed h
            nc.vector.memset(h_alt, 0.0)
            zero1f = st.tile([1, 1], F32)
            nc.vector.memset(zero1f, 0.0)

            def emit_x(s, xbuf, ps3):
                """X-contribution for step s: K=1 ones matmuls starting the
                f+i / g / o PSUM tiles.  Runs in the previous step's tail."""
                xoff = s * 1024
                ps_fi, ps_g, ps_o = ps3
                for dst, c0, c1 in ((ps_fi, 0, 512), (ps_g, 512, 768),
                                    (ps_o, 768, 1024)):
                    for q in range(Q):
                        nc.tensor.matmul(
                            dst[32 * q:32 * q + 32, 0:c1 - c0],
                            ones32[32 * q:32 * q + 1, :],
                            xbuf[32 * q:32 * q + 1, xoff + c0: xoff + c1],
                            start=True, stop=False,
                            skip_group_check=True,
                            tile_position=(32 * q, 32 * q))

            def emit_fill(n):
                """n dummy N=512 matmuls (zero stationary, wh rows as moving)
                into a scratch PSUM row: useless arithmetic whose only job is
                to keep the PE HAM-busy so the clock stays at 2.4 GHz."""
                if n <= 0:
                    return
                fp = pst.tile([1, 512], F32, tag="fill")
                for i in range(n):
                    # one accumulation group: no per-MM PSUM drain between
                    nc.tensor.matmul(fp[0:1, :], zero1[0:1, 0:1],
                                     wh[0:1, i * 512:(i + 1) * 512],
                                     start=(i == 0), stop=(i == n - 1),
                                     skip_group_check=True)

            # k-chunk issue order: evens (h_col cols 0..3) first, odds
            # (cols 4..7) second -- the next step's matvec can then begin
            # as soon as the first half of h_col is written.
            KORD = [0, 2, 4, 6, 1, 3, 5, 7]

            def emit_step(s, xbuf, hring, ps3, ps3_next):
                """one LSTM step; ps3 pre-started with X; ps3_next gets the
                next step's X matmuls during this step's tail."""
                mm_only = variant in ("mm_only",)
                mm_act = variant in ("mm_act",)
                ps_fi, ps_g, ps_o = ps3
                # --- recurrent matvec, interleaved across the 4 col groups,
                # in three column blocks: [f+i 0:512] [g 512:768] [o 768:1024]
                # each into ITS OWN psum tile, so sf/si fire mid-matvec and
                # the whole c chain hides under the g and o blocks.
                for dst, blk_lo, blk_hi in ((ps_fi, 0, 512), (ps_g, 512, 768),
                                            (ps_o, 768, 1024)):
                    for k in KORD:
                        jj = kcol_of_chunk(k)
                        for q in range(Q):
                            nc.tensor.matmul(
                                dst[32 * q:32 * q + 1, 0:blk_hi - blk_lo],
                                h_col[:, jj:jj + 1],
                                wh[:, k * G + q * 1024 + blk_lo:
                                   k * G + q * 1024 + blk_hi],
                                start=False, stop=(k == KORD[-1]),
                                skip_group_check=True,
                                tile_position=(0, 32 * q))
                # next step's X matmuls: issued now, they stream during this
                # step's ACT/DVE tail while the PE would otherwise idle
                if ps3_next is not None:
                    emit_x(s + 1, xbuf, ps3_next)
                if mm_only:
                    return
                # --- gate nonlinearities; per-quarter col order [f|i|g|o].
                # sig(f)|sig(i) is ONE wide ACT; so is split in halves so the
                # first o-selector can issue after only 128 columns.
                nc.scalar.activation(sfsi[:, :], ps_fi[:, 0:2 * S], AF.Sigmoid)
                nc.scalar.activation(ctg[:, S:2 * S], ps_g[:, 0:S], AF.Tanh)
                nc.scalar.activation(so[:, 0:128], ps_o[:, 0:128], AF.Sigmoid)
                nc.scalar.activation(so[:, 128:256], ps_o[:, 128:256],
                                     AF.Sigmoid)
                if mm_act:
                    return
                if variant == "mm_dve":
                    nc.vector.tensor_copy(th[:, 0:128], sfsi[:, 0:128])
                    nc.vector.tensor_copy(hm[:, 0:128], th[:, 0:128])
                    fp2 = pst.tile([1, 512], F32, tag="fillv")
                    nc.tensor.matmul(fp2[0:1, 0:128], zero1[0:1, 0:1],
                                     hm[0:1, 0:128], start=True, stop=True,
                                     skip_group_check=True)
                    return
                emit_fill(FILL_MID)
                # --- c update (row-land), two wide DVE ops; fires under the
                # g/o matvec blocks: uv = [f*c | i*tanh(g)], c = f*c + i*g ---
                nc.vector.tensor_mul(uv[:, :], sfsi[:, :], ctg[:, :])
                nc.vector.tensor_add(ctg[:, 0:S], uv[:, 0:S], uv[:, S:2 * S])
                if variant == "t_cupd":
                    fp2 = pst.tile([1, 512], F32, tag="fillv")
                    nc.tensor.matmul(fp2[0:1, 0:128], zero1f[0:1, 0:1],
                                     ctg[0:1, 0:128], start=True, stop=True,
                                     skip_group_check=True)
                    return
                # --- column-land transposes via selector matmuls.
                # pt_c[p, n]  = c[256n + p], pt_c[p, 4+n] = c[256n + 128 + p]
                # (same h_col layout: col j*4+c = hid chunk 2c+j); ditto pt_o.
                # c's selectors run right after the o block; so halves follow.
                pt_c = pst.tile([128, 8], F32, tag="ptc")
                pt_o = pst.tile([128, 8], F32, tag="pto")
                nc.tensor.matmul(pt_c[:, 0:4], ctg[:, 0:128], self32[:, :],
                                 start=True, stop=True)
                nc.tensor.matmul(pt_c[:, 4:8], ctg[:, 128:256], self32[:, :],
                                 start=True, stop=True)
                nc.tensor.matmul(pt_o[:, 0:4], so[:, 0:128], sel[:, :],
                                 start=True, stop=True)
                nc.tensor.matmul(pt_o[:, 4:8], so[:, 128:256], sel[:, :],
                                 start=True, stop=True)
                # --- h = sig_o * tanh(c) on 8 columns; writes h_col direct --
                hdst = h_alt if variant == "t_selo" else h_col
                nc.scalar.activation(thc[:, 0:4], pt_c[:, 0:4], AF.Tanh)
                nc.scalar.activation(thc[:, 4:8], pt_c[:, 4:8], AF.Tanh)
                nc.vector.tensor_mul(hdst[:, 0:4], pt_o[:, 0:4], thc[:, 0:4])
                nc.vector.tensor_mul(hdst[:, 4:8], pt_o[:, 4:8], thc[:, 4:8])
                emit_fill(FILL_END)
                # --- save h for output phase ---
                nc.vector.tensor_copy(
                    hring.rearrange("p (j s) -> p j s", j=8)[:, :, s],
                    hdst[:, :])

            def emit_body(get_t0):
                """BODY steps; get_t0 = scalar start step (python int or reg)."""
                xbuf = xr.tile([128, RING * 1024], BF16, tag="xring")
                hring = (None if variant in ("mm_only", "mm_act", "mm_dve", "t_cupd")
                         else hrp.tile([128, 8 * BODY], BF16, tag="hring"))
                # refill the ring (BODY steps of X rows, quarter q on part
                # 32q) in 4 chunks so step 0 only waits for the first 8 steps
                xv = xbuf.rearrange("p (t n) -> p t n", t=RING)[::32]
                CH = RING // 4
                for cc in range(4):
                    nc.sync.dma_start(
                        out=xv[:, cc * CH:(cc + 1) * CH, :],
                        in_=X_q[:, bass.ds(get_t0 + cc * CH, CH), :])
                def ps_triple():
                    return (psg.tile([128, 512], F32, tag="psfi", name="psfi"),
                            psg.tile([128, 256], F32, tag="psg", name="psg"),
                            psg.tile([128, 256], F32, tag="pso", name="pso"))
                ps3 = ps_triple()
                emit_x(0, xbuf, ps3)
                for s in range(BODY):
                    ps3_next = ps_triple() if s < BODY - 1 else None
                    emit_step(s, xbuf, hring, ps3, ps3_next)
                    ps3 = ps3_next
                if variant not in ("mm_only", "mm_act", "mm_dve", "t_cupd"):
                    # flush h history
                    nc.sync.dma_start(
                        out=Hh_v[:, :, bass.ds(get_t0, BODY)],
                        in_=hring.rearrange("p (j s) -> p j s", j=8)[:, :, :])

            if use_loop:
                trips = loop_trips if loop_trips is not None else T // BODY
                hint = (mybir.EngineType.PE,)
                stag = (variant == "stag")
                if outer_rep > 1:
                    with tc.For_i(0, outer_rep, 1) as _rep:
                        with tc.For_i(0, trips, 1, hint_engines=hint,
                                      staggered_reset=stag) as it:
                            emit_body(it * BODY)
                else:
                    with tc.For_i(0, trips, 1, hint_engines=hint,
                                  staggered_reset=stag) as it:
                        emit_body(it * BODY)
            else:
                for it in range(T // BODY):
                    emit_body(it * BODY)

        # ---------------- phase 3: output projection ----------------
        with tc.tile_pool(name="p3w", bufs=1) as wpool, \
             tc.tile_pool(name="p3h", bufs=3) as hpool, \
             tc.tile_pool(name="p3o", bufs=4) as opool, \
             tc.tile_pool(name="p3ps", bufs=4, space="PSUM") as pspool, \
             tc.tile_pool(name="p3c", bufs=1) as cpool:
            ow = wpool.tile([128, KC * OUT], BF16)
            for k in range(KC):
                # Hst row-block k holds hid chunk chunk_of_kcol(k); pair the
                # matching out_w^T rows so the contraction lines up.
                ck = chunk_of_kcol(k)
                nc.sync.dma_start(out=ow[:, k * OUT:(k + 1) * OUT],
                                  in_=owT_h[ck * 128:(ck + 1) * 128, :])
            onescol = cpool.tile([1, 128], BF16)
            nc.vector.memset(onescol, 1.0)
            obs = cpool.tile([1, OUT], BF16)
            nc.sync.dma_start(out=obs, in_=ob_h[:, :])

            for tt in range(TT):
                hk = hpool.tile([128, KC * 128], BF16, tag="hk")
                for k in range(KC):
                    nc.sync.dma_start(
                        out=hk[:, k * 128:(k + 1) * 128],
                        in_=Hh_h[k * 128:(k + 1) * 128, tt * 128:(tt + 1) * 128])
                for sl in range(OUT // 512):
                    ps = pspool.tile([128, 512], F32, tag="ps3")
                    nc.tensor.matmul(ps[:, :], onescol[0:1, :],
                                     obs[0:1, sl * 512:(sl + 1) * 512],
                                     start=True, stop=False)
                    for k in range(KC):
                        nc.tensor.matmul(
                            ps[:, :], hk[:, k * 128:(k + 1) * 128],
                            ow[:, k * OUT + sl * 512: k * OUT + (sl + 1) * 512],
                            start=False, stop=(k == KC - 1))
                    ot = opool.tile([128, 512], F32, tag="ot")
                    nc.vector.tensor_copy(ot[:, :], ps[:, :])
                    nc.sync.dma_start(
                        out=Y_h[tt * 128:(tt + 1) * 128, sl * 512:(sl + 1) * 512],
                        in_=ot[:, :])

    return nc


def ref_lstm(x, W_w, W_b, out_w, out_b):
    T = x.shape[0]
    x2 = x.reshape(T, IN).astype(np.float64)
    Wx = W_w[:, :IN].astype(np.float64)
    Wh = W_w[:, IN:].astype(np.float64)
    b = W_b.astype(np.float64)
    h = np.zeros(H); c = np.zeros(H)
    ys = np.zeros((T, OUT))
    sig = lambda v: 1.0 / (1.0 + np.exp(-v))
    for t in range(T):
        g = Wx @ x2[t] + Wh @ h + b
        i_, f_, g_, o_ = g[:H], g[H:2*H], g[2*H:3*H], g[3*H:]
        c = sig(f_) * c + sig(i_) * np.tanh(g_)
        h = sig(o_) * np.tanh(c)
        ys[t] = out_w.astype(np.float64) @ h + out_b.astype(np.float64)
    return ys

_NC_CACHE = None
T_FULL = 8192


def kernel(x, W_w, W_b, out_w, out_b):
    """Full unsharded inputs in; full [8192, 1, 1024] float32 output."""
    global _NC_CACHE
    if _NC_CACHE is None:
        _NC_CACHE = build_nc(T_FULL, BODY=32, use_loop=True)
    prep = host_prep(x, W_w, W_b, out_w, out_b, T_FULL)
    res = run_bass_kernel_spmd(_NC_CACHE, [prep], core_ids=[0])
    return np.asarray(res.results[0]["Y"], dtype=np.float32).reshape(T_FULL, 1, OUT)

